# revision 2
# baseline (speedup 1.0000x reference)
"""Haar DWT decoder (2-level inverse, zero details) as a Trainium2 Bass kernel.

out[b, c, j, k] = z[b].reshape(C, 128, 128)[c, j//4, k//4] * 0.25
i.e. a 4x4 nearest-neighbor upsample scaled by 1/4.

Data-parallel over batch: 128 samples -> 16 per core on 8 NeuronCores.

Per-core shape of the problem: read 3 MiB of z, write 48 MiB of output.
The 16 SDMA engines stream ~26.5 GB/s each (~424 GB/s aggregate), so the
floor is ~126 us of pure streaming; everything else is pipeline lead-in.
The kernel therefore optimizes the head of the pipeline:
  - the three height-replication copies run on three different engines
    (scalar / vector / gpsimd) so they overlap instead of serializing,
  - the first two samples are processed per-channel (1 MiB stores) so both
    HWDGE store rings start streaming as early as possible,
  - input loads stay on the gpsimd SWDGE queue (off the two store rings)
    and are interleaved with gpsimd's copy work.
"""

import numpy as np

import concourse.bass as bass
import concourse.mybir as mybir
import concourse.tile as tile
from concourse.bass_utils import run_bass_kernel_spmd

# The walrus build in this container rejects instructions carrying more than
# one sync-wait command (codegen: "Too many sync wait commands" — observed on
# a Drain with 3 waits and a DMACopy with 2). Tile freely attaches several
# waits to one instruction, so after tracing we split the excess onto NOPs
# inserted just before the instruction on the same engine; sequential
# dispatch on one engine makes that equivalent.
_MAX_WAITS = 1


def _split_excess_waits(nc: bass.Bass) -> None:
    for f in nc.m.functions:
        for bb in f.blocks:
            insns = bb.instructions
            # Iterate over a snapshot; mutate the live list via insert.
            for ins in list(insns):
                si = ins.sync_info
                if si is None or not si.on_wait or len(si.on_wait) <= _MAX_WAITS:
                    continue
                waits = list(si.on_wait)
                keep = waits[-_MAX_WAITS:]
                spill = waits[:-_MAX_WAITS]
                pos = insns.index(ins)
                nops = []
                for i in range(0, len(spill), _MAX_WAITS):
                    nop = nc.engines[ins.engine].nop(nofuse=True).ins
                    # nop() appended itself to the current bb; pull it out.
                    cur = nc.cur_bb.bb.instructions
                    assert cur[-1] is nop
                    cur.pop()
                    nop.sync_info = mybir.SyncInfo(
                        on_wait=spill[i : i + _MAX_WAITS], on_update=[]
                    )
                    nops.append(nop)
                insns[pos:pos] = nops
                ins.sync_info = mybir.SyncInfo(
                    on_wait=keep, on_update=list(si.on_update)
                )

# Problem constants (hardcoded: module config out_shape=(3,512,512), levels=2)
BATCH = 128
C = 3
CAH = 128  # coarse-approximation spatial dims
CAW = 128
S = 4      # 2**levels upsample factor
H = 512
W = 512
N_CORES = 8
B_SHARD = BATCH // N_CORES  # 16

# First SPLIT samples are processed per-channel to shorten the pipeline
# lead-in; PRELOAD loads are issued before the compute loop (must be <=
# zin bufs), the rest are interleaved one-per-sample with gpsimd's copies.
SPLIT = 2
PRELOAD = 6

F32 = mybir.dt.float32


def _build_nc(b_shard: int = B_SHARD) -> bass.Bass:
    nc = bass.Bass("TRN2", target_bir_lowering=False, debug=False)
    z = nc.dram_tensor("z", [b_shard, C * CAH * CAW], F32, kind="ExternalInput").ap()
    out = nc.dram_tensor("out", [b_shard, C, H, W], F32, kind="ExternalOutput").ap()

    with tile.TileContext(nc) as tc:
        with (
            tc.tile_pool(name="zin", bufs=PRELOAD) as zin_pool,
            tc.tile_pool(name="wide", bufs=6) as w_pool,
        ):
            dma_idx = 0
            zts: list = []

            def issue_load(b: int) -> None:
                # Load z[b] as [jc=128 partitions, (c, kc) free] via SWDGE
                # (gpsimd): the HWDGE rings execute FIFO per ring, so loads
                # there would queue behind multi-MiB output DMAs and stall
                # the pipeline. The first SPLIT samples load per-channel so
                # sample 0 / channel 0 is available as early as possible.
                zt = zin_pool.tile([CAH, C * CAW], F32)
                zts.append(zt)
                src = z[b].rearrange("(c jc kc) -> jc c kc", c=C, jc=CAH, kc=CAW)
                dst = zt[:].rearrange("p (c kc) -> p c kc", c=C)
                if b < SPLIT:
                    for c in range(C):
                        nc.gpsimd.dma_start(out=dst[:, c, :], in_=src[:, c, :])
                else:
                    nc.gpsimd.dma_start(out=dst, in_=src)

            for b in range(min(PRELOAD, b_shard)):
                issue_load(b)

            for b in range(b_shard):
                zt = zts[b]
                zv = zt[:].rearrange("p (c kc) -> p c kc", c=C)

                # Materialize the upsampled sample in SBUF: partition jc holds
                # output rows 4*jc..4*jc+3 of every channel, free layout
                # (c, jr, k), so output DMAs are fully contiguous with 8 KiB
                # descriptor runs.
                w2 = w_pool.tile([CAH, C * S * W], F32, tag="wide")
                w2v = w2[:].rearrange(
                    "p (c jr kc kr) -> p c jr kc kr", c=C, jr=S, kc=CAW, kr=S
                )
                w2f = w2[:].rearrange("p (c jr k) -> p c jr k", c=C, jr=S)
                w2c = w2[:].rearrange("p (c jrk) -> p c jrk", c=C)
                ov = out[b].rearrange("c (jc jr) k -> jc c (jr k)", jr=S)

                if b < SPLIT:
                    # Channel-split: three 1 MiB stores per sample; the first
                    # one launches ~5 us earlier than a whole-sample store
                    # would, and the alternation warms up both HWDGE rings.
                    for c in range(C):
                        zb = zv[:, c, :].unsqueeze(2).broadcast_to([CAH, CAW, S])
                        nc.vector.tensor_scalar_mul(w2v[:, c, 0, :, :], zb, 0.25)
                        nc.scalar.copy(w2f[:, c, 1, :], w2f[:, c, 0, :])
                        nc.vector.tensor_copy(w2f[:, c, 2, :], w2f[:, c, 0, :])
                        nc.gpsimd.tensor_copy(w2f[:, c, 3, :], w2f[:, c, 0, :])
                        eng = nc.sync if dma_idx % 2 == 0 else nc.scalar
                        dma_idx += 1
                        eng.dma_start(out=ov[:, c, :], in_=w2c[:, c, :])
                else:
                    # Width-expand x4 (with the 1/4 scale) into the jr=0 rows
                    # in a single contiguous-write op via a 0-stride
                    # (broadcast) input; height-replicate into jr=1..3 split
                    # across scalar, vector and gpsimd so the three copies
                    # overlap (~1.6 us) instead of serializing on one engine.
                    zb = zv.unsqueeze(3).broadcast_to([CAH, C, CAW, S])
                    nc.vector.tensor_scalar_mul(w2v[:, :, 0, :, :], zb, 0.25)
                    nc.scalar.copy(w2f[:, :, 1, :], w2f[:, :, 0, :])
                    nc.vector.tensor_copy(w2f[:, :, 2, :], w2f[:, :, 0, :])
                    nc.gpsimd.tensor_copy(w2f[:, :, 3, :], w2f[:, :, 0, :])

                    # One fully-contiguous 3 MiB DMA per sample; alternate
                    # between the two HWDGE rings for descriptor-gen overlap.
                    # (Splitting steady-state DMAs per-channel measurably
                    # depresses the SDMA rate — keep them whole.)
                    eng = nc.sync if dma_idx % 2 == 0 else nc.scalar
                    dma_idx += 1
                    eng.dma_start(
                        out=ov, in_=w2[:].rearrange("p (c jrk) -> p c jrk", c=C)
                    )

                # Interleave the remaining loads with the compute loop so
                # gpsimd alternates copy / descriptor-gen instead of doing
                # all 16 gens up front (which would delay its copies).
                if b + PRELOAD < b_shard:
                    issue_load(b + PRELOAD)

    _split_excess_waits(nc)
    return nc


_NC_CACHE: dict[int, bass.Bass] = {}


def _get_nc(b_shard: int = B_SHARD) -> bass.Bass:
    if b_shard not in _NC_CACHE:
        _NC_CACHE[b_shard] = _build_nc(b_shard)
    return _NC_CACHE[b_shard]


def kernel(z: np.ndarray) -> np.ndarray:
    z = np.ascontiguousarray(z, dtype=np.float32)
    assert z.shape == (BATCH, C * CAH * CAW), z.shape
    nc = _get_nc()
    in_maps = [
        {"z": z[i * B_SHARD : (i + 1) * B_SHARD]} for i in range(N_CORES)
    ]
    res = run_bass_kernel_spmd(nc, in_maps, list(range(N_CORES)))
    return np.concatenate([res.results[i]["out"] for i in range(N_CORES)], axis=0)
